# revision 14
# baseline (speedup 1.0000x reference)
"""Trainium2 Bass kernel for nn_BatchedNLM.

Per-neuron batched MLP:
    x1 = einsum('bnm,nmh->bnh', state, w1) + b1      # (B, N, 256)
    g1 = glu(x1)                                      # (B, N, 128)
    x2 = einsum('bnh,nho->bno', g1, w2) + b2          # (B, N, 2)
    out = glu(x2)[..., 0] / T                         # (B, N)

Sharding: neuron dimension split across 8 cores (256 neurons/core), no
communication.  Inside each core, per neuron:
  fc1:  matmul(out=[h,b], lhsT=w1[n] [m,h_chunk], rhs=stateT[n] [m,b])
        -> psum in [h, b] layout, two 128-col h-chunks (a-half, gate-half)
  GLU1: ACT sigmoid (PSUM->SBUF) + DVE multiply, batched 4 neurons/psum bank
  fc2:  matmul(out=[b, 2], lhsT=glu [h,b], rhs=w2[n] [h,2]) packed into one
        resident psum bank holding all 256 neurons' (a,gate) column pairs
  GLU2: one strided sigmoid + multiply over the packed [b, 2*256] bank
The output is produced directly in [b, n_local] layout.

Matmul operands are bf16 (fp32 matmul on TRN2 runs as 2 half-rate passes,
~8x slower); PSUM accumulation and everything after the matmuls is fp32.

Two device-program variants:
  fast (biases all zero, the graded case): K=32 contraction; 4 neurons
       stacked on the 128 SBUF partitions (full-bandwidth DMA) with
       tile_position row-group matmuls.
  aug  (any nonzero bias): K=33 with a ones-row appended to stateT and the
       bias row appended to w1, exact bias handling; fc2 bias added via a
       broadcast DVE add before GLU2.
1/T is folded into w2[:, :, 0] (and b2[:, 0]) on the host.
"""

import numpy as np
from contextlib import ExitStack

B = 128        # batch
N = 2048       # neurons
M = 32         # memory (fc1 contraction)
H = 256        # fc1 output width (GLU halves of 128)
NCORES = 8
NPC = N // NCORES   # neurons per core
CH = 32             # neurons per DMA chunk
G1 = 4              # neurons per GLU1 psum group ([128, 512] = one bank)

_cache = {}


def _build(aug: bool, dt_name: str):
    import concourse.mybir as mybir
    import concourse.tile as tile
    from concourse import bacc

    f32 = mybir.dt.float32
    dt_in = getattr(mybir.dt, dt_name)
    Sig = mybir.ActivationFunctionType.Sigmoid
    K = 33 if aug else 32
    KP = K if aug else 128          # partition count of the input tiles

    nc = bacc.Bacc("TRN2", target_bir_lowering=False, debug=False,
                   num_devices=NCORES)

    # m-major layouts: per-partition runs are contiguous across neurons
    if aug:
        state_d = nc.dram_tensor("state", [K, NPC, B], dt_in, kind="ExternalInput")
        w1_d = nc.dram_tensor("w1", [K, NPC, H], dt_in, kind="ExternalInput")
    else:
        # 4 neurons stacked along partitions
        state_d = nc.dram_tensor("state", [128, NPC // 4, B], dt_in, kind="ExternalInput")
        w1_d = nc.dram_tensor("w1", [128, NPC // 4, H], dt_in, kind="ExternalInput")
    w2_d = nc.dram_tensor("w2", [128, NPC * 2], dt_in, kind="ExternalInput")
    if aug:
        b2r_d = nc.dram_tensor("b2r", [128, NPC * 2], f32, kind="ExternalInput")
    out_d = nc.dram_tensor("out", [B, NPC], f32, kind="ExternalOutput")

    with ExitStack() as ctx:
        tc = ctx.enter_context(tile.TileContext(nc))
        sp = ctx.enter_context(tc.tile_pool(name="sp", bufs=2))
        wp = ctx.enter_context(tc.tile_pool(name="wp", bufs=2))
        cp = ctx.enter_context(tc.tile_pool(name="cp", bufs=1))
        sgp = ctx.enter_context(tc.tile_pool(name="sgp", bufs=4))
        glp = ctx.enter_context(tc.tile_pool(name="glp", bufs=6))
        fin = ctx.enter_context(tc.tile_pool(name="fin", bufs=1))
        pap = ctx.enter_context(tc.tile_pool(name="pap", bufs=3, space="PSUM"))
        pgp = ctx.enter_context(tc.tile_pool(name="pgp", bufs=3, space="PSUM"))
        p2p = ctx.enter_context(tc.tile_pool(name="p2p", bufs=1, space="PSUM"))

        w2_sb = cp.tile([128, NPC * 2], dt_in)
        nc.sync.dma_start(out=w2_sb[:], in_=w2_d[:])
        if aug:
            b2r_sb = cp.tile([128, NPC * 2], f32)
            nc.sync.dma_start(out=b2r_sb[:], in_=b2r_d[:])

        # one resident bank collecting every neuron's fc2 (a, gate) pair
        ps2 = p2p.tile([128, NPC * 2], f32)

        # HAM warmup: ~4 us of dense dummy matmuls while the first chunk's
        # DMA is in flight, so the PE clock un-throttles (1.2 -> 2.4 GHz)
        # before real work starts.  Runs on zeroed SBUF; the psum slot is
        # recycled by the pool afterwards.
        warm = cp.tile([33, 128], dt_in)
        nc.vector.memset(warm[:], 0.0)
        wps = pap.tile([128, G1 * B], f32, tag="pa")
        for i in range(48):
            nc.tensor.matmul(wps[:, (i % 4) * 128:(i % 4 + 1) * 128],
                             warm[:], warm[:], start=True, stop=True)

        def emit_fc2(gl, nl0):
            for j in range(G1):
                nl = nl0 + j  # neuron within core
                nc.tensor.matmul(ps2[:, 2 * nl:2 * nl + 2],
                                 gl[:, j * B:(j + 1) * B],
                                 w2_sb[:, 2 * nl:2 * nl + 2],
                                 start=True, stop=True)

        FC2_LAG = 2  # groups of fc2 kept pending so PE never starves
        pend = []    # [(gl, nl0), ...]
        nch = CH if aug else CH // 4  # chunk extent in the middle dram dim
        for ci in range(NPC // CH):
            st = sp.tile([KP, nch, B], dt_in)
            nc.sync.dma_start(out=st[:], in_=state_d[:, ci * nch:(ci + 1) * nch, :])
            wt = wp.tile([KP, nch, H], dt_in)
            nc.sync.dma_start(out=wt[:], in_=w1_d[:, ci * nch:(ci + 1) * nch, :])

            for g in range(CH // G1):
                if len(pend) >= FC2_LAG:
                    emit_fc2(*pend.pop(0))
                pa = pap.tile([128, G1 * B], f32)
                pg = pgp.tile([128, G1 * B], f32)
                # all 4 a-half matmuls, then all 4 gate-half matmuls, so
                # consecutive matmuls hit the same PSUM bank (bank
                # alternation costs a micro-stall per matmul)
                for half, dst in ((0, pa), (128, pg)):
                    for j in range(G1):
                        ns = g * G1 + j  # neuron within chunk
                        if aug:
                            lhsT = wt[:, ns, half:half + 128]
                            rhs = st[:, ns, :]
                            tp = None
                        else:
                            q, r = divmod(ns, 4)
                            lhsT = wt[32 * r:32 * r + 32, q, half:half + 128]
                            rhs = st[32 * r:32 * r + 32, q, :]
                            tp = (32 * r, 0)
                        nc.tensor.matmul(dst[:, j * B:(j + 1) * B], lhsT, rhs,
                                         start=True, stop=True, tile_position=tp)
                sg = sgp.tile([128, G1 * B], f32)
                nc.scalar.activation(sg[:], pg[:], Sig)
                gl = glp.tile([128, G1 * B], dt_in)
                nc.vector.tensor_mul(gl[:], pa[:], sg[:])
                pend.append((gl, ci * CH + g * G1))
        for args in pend:
            emit_fc2(*args)

        if aug:
            fs = fin.tile([128, NPC * 2], f32)
            nc.vector.tensor_add(fs[:], ps2[:], b2r_sb[:])
            src = fs[:].rearrange("p (n o) -> p n o", o=2)
        else:
            src = ps2[:].rearrange("p (n o) -> p n o", o=2)
        s2 = fin.tile([128, NPC], f32)
        nc.scalar.activation(s2[:], src[:, :, 1], Sig)
        ot = fin.tile([128, NPC], f32)
        nc.vector.tensor_mul(ot[:], src[:, :, 0], s2[:])
        nc.sync.dma_start(out=out_d[:], in_=ot[:])

    nc.compile()
    return nc


def _build_pair(dt_name: str):
    """Pair variant: neuron pairs stacked at partition bases 0 / 64 (both
    32-aligned, so matmul row-group auto-derive applies), K=33 with the
    ones/bias augmentation rows (exact for any bias).  State/w1 are loaded
    with two concurrent HWDGE rings (nc.sync -> partitions 0-32 on even
    SBUF ports, nc.scalar -> partitions 64-96 on odd ports) for full DMA
    bandwidth."""
    import concourse.mybir as mybir
    import concourse.tile as tile
    from concourse import bacc

    f32 = mybir.dt.float32
    dt_in = getattr(mybir.dt, dt_name)
    Sig = mybir.ActivationFunctionType.Sigmoid
    K = 33

    nc = bacc.Bacc("TRN2", target_bir_lowering=False, debug=False,
                   num_devices=NCORES)

    NH = NPC // 2  # even/odd halves
    se_d = nc.dram_tensor("se", [K, NH, B], dt_in, kind="ExternalInput")
    so_d = nc.dram_tensor("so", [K, NH, B], dt_in, kind="ExternalInput")
    we_d = nc.dram_tensor("we", [K, NH, H], dt_in, kind="ExternalInput")
    wo_d = nc.dram_tensor("wo", [K, NH, H], dt_in, kind="ExternalInput")
    w2_d = nc.dram_tensor("w2", [128, NPC * 2], dt_in, kind="ExternalInput")
    b2r_d = nc.dram_tensor("b2r", [128, NPC * 2], f32, kind="ExternalInput")
    out_d = nc.dram_tensor("out", [B, NPC], f32, kind="ExternalOutput")

    with ExitStack() as ctx:
        tc = ctx.enter_context(tile.TileContext(nc))
        sp = ctx.enter_context(tc.tile_pool(name="sp", bufs=2))
        wp = ctx.enter_context(tc.tile_pool(name="wp", bufs=2))
        cp = ctx.enter_context(tc.tile_pool(name="cp", bufs=1))
        sgp = ctx.enter_context(tc.tile_pool(name="sgp", bufs=6))
        glp = ctx.enter_context(tc.tile_pool(name="glp", bufs=8))
        fin = ctx.enter_context(tc.tile_pool(name="fin", bufs=1))
        pap = ctx.enter_context(tc.tile_pool(name="pap", bufs=4, space="PSUM"))
        pgp = ctx.enter_context(tc.tile_pool(name="pgp", bufs=3, space="PSUM"))
        p2p = ctx.enter_context(tc.tile_pool(name="p2p", bufs=1, space="PSUM"))

        w2_sb = cp.tile([128, NPC * 2], dt_in)
        nc.sync.dma_start(out=w2_sb[:], in_=w2_d[:])
        b2r_sb = cp.tile([128, NPC * 2], f32)
        nc.sync.dma_start(out=b2r_sb[:], in_=b2r_d[:])

        ps2 = p2p.tile([128, NPC * 2], f32)

        # HAM warmup under the first chunk's DMA
        warm = cp.tile([33, 128], dt_in)
        nc.vector.memset(warm[:], 0.0)
        wps = pap.tile([128, G1 * B], f32, tag="pa")
        for i in range(48):
            nc.tensor.matmul(wps[:, (i % 4) * 128:(i % 4 + 1) * 128],
                             warm[:], warm[:], start=True, stop=True)

        def emit_fc2(gl, nl0):
            for j in range(G1):
                nl = nl0 + j
                nc.tensor.matmul(ps2[:, 2 * nl:2 * nl + 2],
                                 gl[:, j * B:(j + 1) * B],
                                 w2_sb[:, 2 * nl:2 * nl + 2],
                                 start=True, stop=True)

        def emit_fc2_strided(gl, nl0, stride):
            for j in range(G1):
                nl = nl0 + stride * j
                nc.tensor.matmul(ps2[:, 2 * nl:2 * nl + 2],
                                 gl[:, j * B:(j + 1) * B],
                                 w2_sb[:, 2 * nl:2 * nl + 2],
                                 start=True, stop=True)

        FC2_LAG = 4
        pend = []
        CHP = 32       # neurons per DMA chunk (CH=64 measured slower: 99.3 vs 95.9 us)
        CH2 = CHP // 2  # pairs per chunk
        for ci in range(NPC // CHP):
            st = sp.tile([97, CH2, B], dt_in)
            nc.sync.dma_start(out=st[0:33, :, :],
                              in_=se_d[:, ci * CH2:(ci + 1) * CH2, :])
            nc.scalar.dma_start(out=st[64:97, :, :],
                                in_=so_d[:, ci * CH2:(ci + 1) * CH2, :])
            wt = wp.tile([97, CH2, H], dt_in)
            nc.sync.dma_start(out=wt[0:33, :, :],
                              in_=we_d[:, ci * CH2:(ci + 1) * CH2, :])
            nc.scalar.dma_start(out=wt[64:97, :, :],
                                in_=wo_d[:, ci * CH2:(ci + 1) * CH2, :])

            # super-groups of 8 neurons: 4 even (partitions 0-32) and 4 odd
            # (partitions 64-96).  Even/odd matmuls are interleaved so
            # consecutive matmuls hit different PE row groups (subarray
            # concurrency) and different PSUM banks.
            for s in range(CHP // 8):
                while len(pend) >= FC2_LAG:
                    emit_fc2_strided(*pend.pop(0))
                pae = pap.tile([128, G1 * B], f32, tag="pa")
                pao = pap.tile([128, G1 * B], f32, tag="pa")
                pge = pgp.tile([128, G1 * B], f32, tag="pg")
                pgo = pgp.tile([128, G1 * B], f32, tag="pg")
                q0 = s * 4  # first pair index of this super-group
                for half, de, do in ((0, pae, pao), (128, pge, pgo)):
                    for j in range(G1):
                        q = q0 + j
                        nc.tensor.matmul(de[:, j * B:(j + 1) * B],
                                         wt[0:33, q, half:half + 128],
                                         st[0:33, q, :],
                                         start=True, stop=True)
                        nc.tensor.matmul(do[:, j * B:(j + 1) * B],
                                         wt[64:97, q, half:half + 128],
                                         st[64:97, q, :],
                                         start=True, stop=True)
                sge = sgp.tile([128, G1 * B], f32, tag="sg")
                nc.scalar.activation(sge[:], pge[:], Sig)
                sgo = sgp.tile([128, G1 * B], f32, tag="sg")
                nc.scalar.activation(sgo[:], pgo[:], Sig)
                gle = glp.tile([128, G1 * B], dt_in, tag="gl")
                nc.vector.tensor_mul(gle[:], pae[:], sge[:])
                glo = glp.tile([128, G1 * B], dt_in, tag="gl")
                nc.vector.tensor_mul(glo[:], pao[:], sgo[:])
                nl0 = ci * CHP + s * 8
                pend.append((gle, nl0, 2))      # even neurons nl0, nl0+2, ...
                pend.append((glo, nl0 + 1, 2))  # odd neurons nl0+1, nl0+3, ...
        for args in pend:
            emit_fc2_strided(*args)

        fs = fin.tile([128, NPC * 2], f32)
        nc.vector.tensor_add(fs[:], ps2[:], b2r_sb[:])
        src = fs[:].rearrange("p (n o) -> p n o", o=2)
        s2 = fin.tile([128, NPC], f32)
        nc.scalar.activation(s2[:], src[:, :, 1], Sig)
        ot = fin.tile([128, NPC], f32)
        nc.vector.tensor_mul(ot[:], src[:, :, 0], s2[:])
        nc.sync.dma_start(out=out_d[:], in_=ot[:])

    nc.compile()
    return nc


def _build_quad(dt_name: str):
    """Zero-bias variant: K=32, four consecutive neurons stacked on the 128
    partitions (row groups 0-3), matmuls interleaved across row groups for
    4-way PE subarray concurrency, two PSUM banks per half (2 writers per
    bank).  Full-partition single-ring DMA with 64-neuron chunks."""
    import concourse.mybir as mybir
    import concourse.tile as tile
    from concourse import bacc

    f32 = mybir.dt.float32
    dt_in = getattr(mybir.dt, dt_name)
    Sig = mybir.ActivationFunctionType.Sigmoid
    CHQ = 64  # neurons per DMA chunk

    nc = bacc.Bacc("TRN2", target_bir_lowering=False, debug=False,
                   num_devices=NCORES)

    state_d = nc.dram_tensor("state", [128, NPC // 4, B], dt_in, kind="ExternalInput")
    w1_d = nc.dram_tensor("w1", [128, NPC // 4, H], dt_in, kind="ExternalInput")
    w2_d = nc.dram_tensor("w2", [128, NPC * 2], dt_in, kind="ExternalInput")
    out_d = nc.dram_tensor("out", [B, NPC], f32, kind="ExternalOutput")

    with ExitStack() as ctx:
        tc = ctx.enter_context(tile.TileContext(nc))
        sp = ctx.enter_context(tc.tile_pool(name="sp", bufs=2))
        wp = ctx.enter_context(tc.tile_pool(name="wp", bufs=2))
        cp = ctx.enter_context(tc.tile_pool(name="cp", bufs=1))
        sgp = ctx.enter_context(tc.tile_pool(name="sgp", bufs=6))
        glp = ctx.enter_context(tc.tile_pool(name="glp", bufs=8))
        fin = ctx.enter_context(tc.tile_pool(name="fin", bufs=1))
        pap = ctx.enter_context(tc.tile_pool(name="pap", bufs=4, space="PSUM"))
        pgp = ctx.enter_context(tc.tile_pool(name="pgp", bufs=3, space="PSUM"))
        p2p = ctx.enter_context(tc.tile_pool(name="p2p", bufs=1, space="PSUM"))

        w2_sb = cp.tile([128, NPC * 2], dt_in)
        nc.sync.dma_start(out=w2_sb[:], in_=w2_d[:])

        ps2 = p2p.tile([128, NPC * 2], f32)

        warm = cp.tile([33, 128], dt_in)
        nc.vector.memset(warm[:], 0.0)
        wps = pap.tile([128, G1 * B], f32, tag="pa")
        for i in range(48):
            nc.tensor.matmul(wps[:, (i % 4) * 128:(i % 4 + 1) * 128],
                             warm[:], warm[:], start=True, stop=True)

        def emit_fc2_list(gl, nlist):
            for j, nl in enumerate(nlist):
                nc.tensor.matmul(ps2[:, 2 * nl:2 * nl + 2],
                                 gl[:, j * B:(j + 1) * B],
                                 w2_sb[:, 2 * nl:2 * nl + 2],
                                 start=True, stop=True)

        FC2_LAG = 4
        pend = []
        nch = CHQ // 4  # stacked columns per chunk
        for ci in range(NPC // CHQ):
            st = sp.tile([128, nch, B], dt_in)
            nc.sync.dma_start(out=st[:], in_=state_d[:, ci * nch:(ci + 1) * nch, :])
            wt = wp.tile([128, nch, H], dt_in)
            nc.sync.dma_start(out=wt[:], in_=w1_d[:, ci * nch:(ci + 1) * nch, :])

            # super-group: 2 stacked columns = 8 neurons; row groups 0-1 of
            # both columns fill pae, row groups 2-3 fill pao
            for s in range(nch // 2):
                while len(pend) >= FC2_LAG:
                    emit_fc2_list(*pend.pop(0))
                pae = pap.tile([128, G1 * B], f32, tag="pa")
                pao = pap.tile([128, G1 * B], f32, tag="pa")
                pge = pgp.tile([128, G1 * B], f32, tag="pg")
                pgo = pgp.tile([128, G1 * B], f32, tag="pg")
                q0 = s * 2
                ks = (0, 2, 1, 3, 4, 6, 5, 7)  # alternate row groups
                for half, de, do in ((0, pae, pao), (128, pge, pgo)):
                    for k in ks:
                        qd, r = divmod(k, 4)
                        q = q0 + qd
                        dst = de if r < 2 else do
                        cj = 2 * qd + (r % 2)
                        nc.tensor.matmul(dst[:, cj * B:(cj + 1) * B],
                                         wt[32 * r:32 * r + 32, q, half:half + 128],
                                         st[32 * r:32 * r + 32, q, :],
                                         start=True, stop=True,
                                         tile_position=(32 * r, 0))
                sge = sgp.tile([128, G1 * B], f32, tag="sg")
                nc.scalar.activation(sge[:], pge[:], Sig)
                sgo = sgp.tile([128, G1 * B], f32, tag="sg")
                nc.scalar.activation(sgo[:], pgo[:], Sig)
                gle = glp.tile([128, G1 * B], dt_in, tag="gl")
                nc.vector.tensor_mul(gle[:], pae[:], sge[:])
                glo = glp.tile([128, G1 * B], dt_in, tag="gl")
                nc.vector.tensor_mul(glo[:], pao[:], sgo[:])
                n0 = ci * CHQ + s * 8
                pend.append((gle, [n0, n0 + 1, n0 + 4, n0 + 5]))
                pend.append((glo, [n0 + 2, n0 + 3, n0 + 6, n0 + 7]))
        for args in pend:
            emit_fc2_list(*args)

        src = ps2[:].rearrange("p (n o) -> p n o", o=2)
        s2 = fin.tile([128, NPC], f32)
        nc.scalar.activation(s2[:], src[:, :, 1], Sig)
        ot = fin.tile([128, NPC], f32)
        nc.vector.tensor_mul(ot[:], src[:, :, 0], s2[:])
        nc.sync.dma_start(out=out_d[:], in_=ot[:])

    nc.compile()
    return nc


def _build_v3(dt_name: str):
    """Zero-bias block-diagonal variant.

    fc1 per 4-neuron group: ONE 128-row LDWEIGHTS per h-half (dense
    [128,128] = 4 neurons' w1 chunks stacked on the contraction rows) +
    ONE N=512 matmul whose rhs is a block-diagonal state tile (band r of
    the rows holds neuron r's state in column block r, zeros elsewhere,
    so the full-row contraction picks out exactly one neuron per column
    block).  The zeros are memset once into the persistent rhs buffers;
    the per-chunk DMAs overwrite only the (band, block) diagonal.

    This keeps the PE weight-load path off the critical path: measured
    on HW, an N=512 matmul stream (216 ns) fully hides the next 128-col
    LDWEIGHTS (~100 ns).  fc2 keeps the packed ps2 layout; its per-neuron
    gl LDWEIGHTS are interleaved so one of each pair hides under an fc1
    stream.
    """
    import concourse.mybir as mybir
    import concourse.tile as tile
    from concourse import bacc

    f32 = mybir.dt.float32
    dt_in = getattr(mybir.dt, dt_name)
    Sig = mybir.ActivationFunctionType.Sigmoid

    GPC = 8              # groups per chunk
    NCH = (NPC // 4) // GPC  # chunks (64 groups / 8)

    nc = bacc.Bacc("TRN2", target_bir_lowering=False, debug=False,
                   num_devices=NCORES)

    state_d = nc.dram_tensor("state", [128, NPC // 4, B], dt_in, kind="ExternalInput")
    # chunks 0-1 pre-padded to the block-diagonal layout (zeros included) so
    # no on-device memset is needed; later chunks reuse the zero regions.
    stz_d = nc.dram_tensor("stz", [128, 3, GPC, 4, B], dt_in, kind="ExternalInput")
    w1_d = nc.dram_tensor("w1", [128, NPC // 4, H], dt_in, kind="ExternalInput")
    w2_d = nc.dram_tensor("w2", [128, NPC * 2], dt_in, kind="ExternalInput")
    out_d = nc.dram_tensor("out", [B, NPC], f32, kind="ExternalOutput")

    with ExitStack() as ctx:
        tc = ctx.enter_context(tile.TileContext(nc))
        sp = ctx.enter_context(tc.tile_pool(name="sp", bufs=3))
        wp = ctx.enter_context(tc.tile_pool(name="wp", bufs=3))
        cp = ctx.enter_context(tc.tile_pool(name="cp", bufs=1))
        sgp = ctx.enter_context(tc.tile_pool(name="sgp", bufs=4))
        glp = ctx.enter_context(tc.tile_pool(name="glp", bufs=10))
        fin = ctx.enter_context(tc.tile_pool(name="fin", bufs=1))
        pap = ctx.enter_context(tc.tile_pool(name="pap", bufs=4, space="PSUM"))
        pgp = ctx.enter_context(tc.tile_pool(name="pgp", bufs=3, space="PSUM"))
        p2p = ctx.enter_context(tc.tile_pool(name="p2p", bufs=1, space="PSUM"))

        w2_sb = cp.tile([128, NPC * 2], dt_in)

        ps2 = p2p.tile([128, NPC * 2], f32)

        # HAM warmup while the first chunk's DMA is in flight
        warm = cp.tile([33, 128], dt_in)
        nc.vector.memset(warm[:], 0.0)
        wps = pap.tile([128, 512], f32, tag="pa")
        for i in range(40):
            nc.tensor.matmul(wps[:, (i % 4) * 128:(i % 4 + 1) * 128],
                             warm[:], warm[:], start=True, stop=True)

        pend = []  # (gl_tile, first neuron, neurons emitted so far)
        FC2_LAG = 3  # groups of fc2 kept pending

        def emit_fc2_one():
            if not pend:
                return False
            gl, nl0, j = pend[0]
            nc.tensor.matmul(ps2[:, 2 * (nl0 + j):2 * (nl0 + j) + 2],
                             gl[:, j * B:(j + 1) * B],
                             w2_sb[:, 2 * (nl0 + j):2 * (nl0 + j) + 2],
                             start=True, stop=True)
            if j == 3:
                pend.pop(0)
            else:
                pend[0] = (gl, nl0, j + 1)
            return True

        for ci in range(NCH):
            # banded-zero rhs tile: [128, GPC, 4, B]; band r valid only in
            # col block r.  Zeros memset at first use; pool cycling reuses
            # the same two buffers so the zero regions persist.
            st = sp.tile([128, GPC, 4, B], dt_in, tag="st")
            hg = GPC // 2
            if ci < 3:
                # both HWDGE rings in parallel, half the groups each
                nc.sync.dma_start(out=st[:, 0:hg, :, :],
                                  in_=stz_d[:, ci, 0:hg, :, :])
                nc.scalar.dma_start(out=st[:, hg:GPC, :, :],
                                    in_=stz_d[:, ci, hg:GPC, :, :])
            else:
                for r in range(4):
                    eng = nc.sync if r % 2 == 0 else nc.scalar
                    eng.dma_start(
                        out=st[32 * r:32 * r + 32, :, r, :],
                        in_=state_d[32 * r:32 * r + 32, ci * GPC:(ci + 1) * GPC, :])
            wt = wp.tile([128, GPC, H], dt_in, tag="wt")
            nc.sync.dma_start(out=wt[:, 0:hg, :],
                              in_=w1_d[:, ci * GPC:ci * GPC + hg, :])
            nc.scalar.dma_start(out=wt[:, hg:GPC, :],
                                in_=w1_d[:, ci * GPC + hg:(ci + 1) * GPC, :])
            if ci == 0:
                nc.scalar.dma_start(out=w2_sb[:], in_=w2_d[:])

            for g8 in range(GPC):
                g = ci * GPC + g8
                pa = pap.tile([128, 512], f32, tag="pa")
                pg = pgp.tile([128, 512], f32, tag="pg")
                rhs = st[:, g8, :, :]
                nc.tensor.matmul(pg[:], wt[:, g8, 128:256], rhs,
                                 start=True, stop=True)
                if len(pend) >= FC2_LAG:
                    emit_fc2_one()
                    emit_fc2_one()
                nc.tensor.matmul(pa[:], wt[:, g8, 0:128], rhs,
                                 start=True, stop=True)
                if len(pend) >= FC2_LAG:
                    emit_fc2_one()
                    emit_fc2_one()
                sg = sgp.tile([128, 512], dt_in, tag="sg")
                nc.scalar.activation(sg[:], pg[:], Sig)
                gl = glp.tile([128, 512], dt_in, tag="gl")
                nc.vector.tensor_mul(gl[:], pa[:], sg[:])
                pend.append((gl, g * 4, 0))
        while pend:
            emit_fc2_one()

        src = ps2[:].rearrange("p (n o) -> p n o", o=2)
        s2 = fin.tile([128, NPC], f32)
        nc.scalar.activation(s2[:], src[:, :, 1], Sig)
        ot = fin.tile([128, NPC], f32)
        nc.vector.tensor_mul(ot[:], src[:, :, 0], s2[:])
        nc.sync.dma_start(out=out_d[:], in_=ot[:])

    nc.compile()
    return nc


def _run_v3(inputs: dict, dt_name: str = "bfloat16", trace: bool = False):
    from concourse import bass_utils

    in_maps = _prepare_quad(inputs["state_trace"], inputs["fc1_weight"],
                            inputs["fc2_weight"], inputs["T"], dt_name)
    # build the zero-padded block-diagonal tiles for chunks 0-1 per core:
    # stz[p, c, g, r, b] = state[p, c*8+g, b] if p//32 == r else 0
    GPC = 8
    for m in in_maps:
        s = m["state"]                       # [128, NPC//4, B]
        stz = np.zeros((128, 3, GPC, 4, B), dtype=s.dtype)
        for c in range(3):
            for r in range(4):
                stz[32 * r:32 * r + 32, c, :, r, :] = \
                    s[32 * r:32 * r + 32, c * GPC:(c + 1) * GPC, :]
        m["stz"] = stz
    key = ("v3", dt_name)
    if key not in _cache:
        _cache[key] = _build_v3(dt_name)
    nc = _cache[key]
    res = bass_utils.run_bass_kernel_spmd(
        nc, in_maps, core_ids=list(range(NCORES)), trace=trace)
    out = np.concatenate(
        [np.asarray(res.results[c]["out"]) for c in range(NCORES)], axis=1)
    return out.astype(np.float32), res.exec_time_ns


def _prepare_quad(state_trace, fc1_weight, fc2_weight, T, dt_name: str):
    if dt_name == "float32":
        np_dt = np.float32
    else:
        import ml_dtypes
        np_dt = getattr(ml_dtypes, dt_name)

    state_trace = np.asarray(state_trace, dtype=np.float32)
    fc1_weight = np.asarray(fc1_weight, dtype=np.float32)
    fc2_weight = np.asarray(fc2_weight, dtype=np.float32)
    t = float(np.asarray(T).reshape(-1)[0])

    w2f = fc2_weight.copy()
    w2f[:, :, 0] /= t

    stateT = np.ascontiguousarray(state_trace.transpose(1, 2, 0))    # (N,32,B)
    state_in = stateT.reshape(N // 4, 128, B).transpose(1, 0, 2)     # (128,N/4,B)
    w1_in = fc1_weight.reshape(N // 4, 128, H).transpose(1, 0, 2)    # (128,N/4,H)
    w2T = w2f.transpose(1, 0, 2)                                     # (128,N,2)

    state_in = np.ascontiguousarray(state_in).astype(np_dt)
    w1_in = np.ascontiguousarray(w1_in).astype(np_dt)
    w2T = np.ascontiguousarray(w2T).astype(np_dt)

    in_maps = []
    gpc = (N // 4) // NCORES
    for c in range(NCORES):
        n0, n1 = c * NPC, (c + 1) * NPC
        in_maps.append({
            "state": np.ascontiguousarray(state_in[:, c * gpc:(c + 1) * gpc, :]),
            "w1": np.ascontiguousarray(w1_in[:, c * gpc:(c + 1) * gpc, :]),
            "w2": np.ascontiguousarray(w2T[:, n0:n1, :]).reshape(128, NPC * 2),
        })
    return in_maps


def _run_quad(inputs: dict, dt_name: str = "bfloat16", trace: bool = False):
    from concourse import bass_utils

    in_maps = _prepare_quad(inputs["state_trace"], inputs["fc1_weight"],
                            inputs["fc2_weight"], inputs["T"], dt_name)
    key = ("quad", dt_name)
    if key not in _cache:
        _cache[key] = _build_quad(dt_name)
    nc = _cache[key]
    res = bass_utils.run_bass_kernel_spmd(
        nc, in_maps, core_ids=list(range(NCORES)), trace=trace)
    out = np.concatenate(
        [np.asarray(res.results[c]["out"]) for c in range(NCORES)], axis=1)
    return out.astype(np.float32), res.exec_time_ns


def _prepare_pair(state_trace, fc1_weight, fc1_bias, fc2_weight, fc2_bias, T,
                  dt_name: str):
    if dt_name == "float32":
        np_dt = np.float32
    else:
        import ml_dtypes
        np_dt = getattr(ml_dtypes, dt_name)

    state_trace = np.asarray(state_trace, dtype=np.float32)
    fc1_weight = np.asarray(fc1_weight, dtype=np.float32)
    fc1_bias = np.asarray(fc1_bias, dtype=np.float32)
    fc2_weight = np.asarray(fc2_weight, dtype=np.float32)
    fc2_bias = np.asarray(fc2_bias, dtype=np.float32)
    t = float(np.asarray(T).reshape(-1)[0])

    w2f = fc2_weight.copy()
    w2f[:, :, 0] /= t
    b2f = fc2_bias.copy()
    b2f[:, 0] /= t

    stateT = state_trace.transpose(1, 2, 0)                          # (N,32,B)
    state_in = np.concatenate([stateT, np.ones((N, 1, B), np.float32)],
                              axis=1).transpose(1, 0, 2)             # (33,N,B)
    w1_in = np.concatenate([fc1_weight, fc1_bias[:, None, :]],
                           axis=1).transpose(1, 0, 2)                # (33,N,H)
    w2T = w2f.transpose(1, 0, 2)                                     # (128,N,2)

    state_in = np.ascontiguousarray(state_in).astype(np_dt)
    w1_in = np.ascontiguousarray(w1_in).astype(np_dt)
    w2T = np.ascontiguousarray(w2T).astype(np_dt)

    in_maps = []
    for c in range(NCORES):
        n0, n1 = c * NPC, (c + 1) * NPC
        m = {
            "se": np.ascontiguousarray(state_in[:, n0:n1:2, :]),
            "so": np.ascontiguousarray(state_in[:, n0 + 1:n1:2, :]),
            "we": np.ascontiguousarray(w1_in[:, n0:n1:2, :]),
            "wo": np.ascontiguousarray(w1_in[:, n0 + 1:n1:2, :]),
            "w2": np.ascontiguousarray(w2T[:, n0:n1, :]).reshape(128, NPC * 2),
            "b2r": np.ascontiguousarray(
                np.broadcast_to(b2f[n0:n1].reshape(1, NPC * 2), (128, NPC * 2))),
        }
        in_maps.append(m)
    return in_maps


def _run_pair(inputs: dict, dt_name: str = "bfloat16", trace: bool = False):
    from concourse import bass_utils

    in_maps = _prepare_pair(dt_name=dt_name, **inputs)
    key = ("pair", dt_name)
    if key not in _cache:
        _cache[key] = _build_pair(dt_name)
    nc = _cache[key]
    res = bass_utils.run_bass_kernel_spmd(
        nc, in_maps, core_ids=list(range(NCORES)), trace=trace)
    out = np.concatenate(
        [np.asarray(res.results[c]["out"]) for c in range(NCORES)], axis=1)
    return out.astype(np.float32), res.exec_time_ns


def _get_nc(aug: bool, dt_name: str):
    key = (aug, dt_name)
    if key not in _cache:
        _cache[key] = _build(aug, dt_name)
    return _cache[key]


def _prepare(state_trace, fc1_weight, fc1_bias, fc2_weight, fc2_bias, T,
             dt_name: str, override_aug=None):
    """Returns (aug, in_maps) — per-core input dicts."""
    if dt_name == "float32":
        np_dt = np.float32
    else:
        import ml_dtypes
        np_dt = getattr(ml_dtypes, dt_name)

    state_trace = np.asarray(state_trace, dtype=np.float32)
    fc1_weight = np.asarray(fc1_weight, dtype=np.float32)
    fc1_bias = np.asarray(fc1_bias, dtype=np.float32)
    fc2_weight = np.asarray(fc2_weight, dtype=np.float32)
    fc2_bias = np.asarray(fc2_bias, dtype=np.float32)
    t = float(np.asarray(T).reshape(-1)[0])

    aug = bool(np.any(fc1_bias) or np.any(fc2_bias))
    if override_aug is not None:
        aug = bool(override_aug)
        assert aug or not (np.any(fc1_bias) or np.any(fc2_bias))

    # fold 1/T into the linear 'a' path of fc2
    w2f = fc2_weight.copy()
    w2f[:, :, 0] /= t
    b2f = fc2_bias.copy()
    b2f[:, 0] /= t

    stateT = state_trace.transpose(1, 2, 0)                         # (N, 32, B)
    if aug:
        state_in = np.concatenate(
            [stateT, np.ones((N, 1, B), np.float32)], axis=1)       # (N, 33, B)
        w1_in = np.concatenate(
            [fc1_weight, fc1_bias[:, None, :]], axis=1)             # (N, 33, H)
        kp = 33
        state_in = state_in.transpose(1, 0, 2)                      # (33, N, B)
        w1_in = w1_in.transpose(1, 0, 2)                            # (33, N, H)
    else:
        state_in = np.ascontiguousarray(stateT).reshape(N // 4, 128, B)
        w1_in = fc1_weight.reshape(N // 4, 128, H)
        kp = 128
        state_in = state_in.transpose(1, 0, 2)                      # (128, N/4, B)
        w1_in = w1_in.transpose(1, 0, 2)                            # (128, N/4, H)
    w2T = w2f.transpose(1, 0, 2)                                    # (128, N, 2)

    state_in = np.ascontiguousarray(state_in).astype(np_dt)
    w1_in = np.ascontiguousarray(w1_in).astype(np_dt)
    w2T = np.ascontiguousarray(w2T).astype(np_dt)

    in_maps = []
    gpc = state_in.shape[1] // NCORES  # per-core extent of the middle dim
    for c in range(NCORES):
        n0, n1 = c * NPC, (c + 1) * NPC
        m = {
            "state": np.ascontiguousarray(state_in[:, c * gpc:(c + 1) * gpc, :]),
            "w1": np.ascontiguousarray(w1_in[:, c * gpc:(c + 1) * gpc, :]),
            "w2": np.ascontiguousarray(w2T[:, n0:n1, :]).reshape(128, NPC * 2),
        }
        if aug:
            m["b2r"] = np.ascontiguousarray(
                np.broadcast_to(b2f[n0:n1].reshape(1, NPC * 2), (128, NPC * 2)))
        in_maps.append(m)
    return aug, in_maps


def _run(inputs: dict, dt_name: str = "bfloat16", trace: bool = False,
         force_aug=None):
    """Returns (output (B, N) float32, exec_time_ns or None)."""
    from concourse import bass_utils

    aug, in_maps = _prepare(dt_name=dt_name, override_aug=force_aug, **inputs)
    nc = _get_nc(aug, dt_name)
    res = bass_utils.run_bass_kernel_spmd(
        nc, in_maps, core_ids=list(range(NCORES)), trace=trace)
    out = np.concatenate(
        [np.asarray(res.results[c]["out"]) for c in range(NCORES)], axis=1)
    return out.astype(np.float32), res.exec_time_ns


def kernel(**inputs) -> np.ndarray:
    # The K=33 pair variant (even/odd neurons at partition bases 0/64,
    # dual-ring DMA) is exact for any bias values and is the fastest
    # hardware-validated configuration (~96 us, rel err ~4e-3 from bf16
    # matmul operands).
    out, _ = _run_pair(inputs, dt_name="bfloat16")
    return out

